# revision 2
# baseline (speedup 1.0000x reference)
"""Trainium2 Bass kernel for nn_LCAMatrixModel (pairwise selu-MLP grid).

Computes out[i,j] = hard_sigmoid(W2 . selu(A[j] + B[i] + b1) + b2) with
  z = x @ W_enc + b_enc, A = z @ W1[:d], B = z @ W1[d:]
for n=1024, d=128, h=256, distributed over 8 NeuronCores by sharding the
output row dimension i (128 rows per core; x and weights replicated).

Per-core algorithm (all math on device):
  selu(v) = lam*relu(v) + lam*(min(alpha*e^v, alpha) - alpha)
  e^v factorizes: alpha*e^v = P[k,j] * Q[k,i],  P = e^{A^T + ln(alpha)},
  Q = e^{B'^T} with B' = B + b1.  Two fp16 "planes" per (i, k-chunk):
    plane1 = relu(A^T + B'^T[:,i])            (ScalarE, bias-fused)
    plane2 = min(P * Q[:,i], alpha)           (VectorE dual-op tensor_scalar)
  Both are contracted with w = lam*W2/6 on TensorE into a PSUM accumulator
  [128 i, 1024 j] using a sliding-window weight tile (w embedded at column
  128 of a zero [128,256] tile; lhsT window [:,128-i:256-i] routes row i).
  Epilogue: out = min(relu(acc + C), 1), C = 0.5 + (b2 - lam*alpha*sum(W2))/6.
"""

import numpy as np
from contextlib import ExitStack

import concourse.bass as bass
import concourse.bacc as bacc
import concourse.mybir as mybir
from concourse import tile
from concourse import bass_utils

N = 1024
RAW = 128
D = 128
H = 256
N_CORES = 8
IB = N // N_CORES  # 128 output rows per core

LAM = 1.0507009873554804934193349852946
ALPHA = 1.6732632423543772848170429916717

F32 = mybir.dt.float32
F16 = mybir.dt.float16

_CACHE = {}


def build_kernel(n_i=IB):
    AF = mybir.ActivationFunctionType
    OP = mybir.AluOpType

    nc = bacc.Bacc(
        "TRN2",
        target_bir_lowering=False,
        debug=False,
        enable_asserts=False,
        num_devices=N_CORES,
    )
    x_d = nc.dram_tensor("x", [N, RAW], F32, kind="ExternalInput").ap()
    xb_d = nc.dram_tensor("xb", [IB, RAW], F32, kind="ExternalInput").ap()
    we_d = nc.dram_tensor("w_enc", [RAW, D], F32, kind="ExternalInput").ap()
    be_d = nc.dram_tensor("b_enc", [D, 1], F32, kind="ExternalInput").ap()
    w1_d = nc.dram_tensor("w1", [2 * D, H], F32, kind="ExternalInput").ap()
    b1_d = nc.dram_tensor("b1", [H, 1], F32, kind="ExternalInput").ap()
    w2_d = nc.dram_tensor("w2", [H, 1], F32, kind="ExternalInput").ap()
    b2_d = nc.dram_tensor("b2", [1, 1], F32, kind="ExternalInput").ap()
    id_d = nc.dram_tensor("ident", [128, 128], F32, kind="ExternalInput").ap()
    y_d = nc.dram_tensor("y", [IB, N], F32, kind="ExternalOutput").ap()

    with tile.TileContext(nc) as tc, ExitStack() as ctx:
        const = ctx.enter_context(tc.tile_pool(name="const", bufs=1))
        planes = ctx.enter_context(tc.tile_pool(name="planes", bufs=4))
        accp = ctx.enter_context(tc.tile_pool(name="acc", bufs=1, space="PSUM"))

        # ---------------- prologue (inside its own psum pool scope) ---------
        with tc.tile_pool(name="ppsum", bufs=2, space="PSUM") as pp, tc.tile_pool(
            name="ppsum1", bufs=1, space="PSUM"
        ) as pp1:
            ident = const.tile([128, 128], F32, tag="ident")
            nc.sync.dma_start(ident[:], id_d[:])
            wenc = const.tile([128, 128], F32, tag="wenc")
            nc.sync.dma_start(wenc[:], we_d[:])
            benc = const.tile([128, 1], F32, tag="benc")
            nc.sync.dma_start(benc[:], be_d[:])
            w1a = const.tile([128, 256], F32, tag="w1a")
            nc.sync.dma_start(w1a[:], w1_d[0:128, :])
            w1b = const.tile([128, 256], F32, tag="w1b")
            nc.sync.dma_start(w1b[:], w1_d[128:256, :])
            b1t = []
            for c in range(2):
                t = const.tile([128, 1], F32, tag=f"b1_{c}")
                nc.sync.dma_start(t[:], b1_d[c * 128 : (c + 1) * 128, :])
                b1t.append(t)
            w2t = const.tile([128, 2], F32, tag="w2t")
            for c in range(2):
                nc.sync.dma_start(w2t[:, c : c + 1], w2_d[c * 128 : (c + 1) * 128, :])
            b2t = const.tile([1, 1], F32, tag="b2t")
            nc.sync.dma_start(b2t[:], b2_d[:])
            xsb = const.tile([128, 1024], F32, tag="xsb")
            for t in range(8):
                nc.sync.dma_start(
                    xsb[:, t * 128 : (t + 1) * 128], x_d[t * 128 : (t + 1) * 128, :]
                )
            xbsb = const.tile([128, 128], F32, tag="xbsb")
            nc.sync.dma_start(xbsb[:], xb_d[:])

            # transposes: x^T [raw, n], xb^T [raw, ib]
            xT = const.tile([128, 1024], F32, tag="xT")
            for t in range(8):
                ps = pp.tile([128, 128], F32, tag="tps")
                nc.tensor.transpose(ps[:], xsb[:, t * 128 : (t + 1) * 128], ident[:])
                nc.vector.tensor_copy(xT[:, t * 128 : (t + 1) * 128], ps[:])
            xbT = const.tile([128, 128], F32, tag="xbT")
            ps = pp.tile([128, 128], F32, tag="tps")
            nc.tensor.transpose(ps[:], xbsb[:], ident[:])
            nc.vector.tensor_copy(xbT[:], ps[:])

            # z^T = W_enc^T x^T + b_enc  [d, n];  zb^T likewise [d, ib]
            zT = const.tile([128, 1024], F32, tag="zT")
            for jh in range(2):
                ps = pp.tile([128, 512], F32, tag="zps")
                nc.tensor.matmul(
                    ps[:], wenc[:], xT[:, jh * 512 : (jh + 1) * 512],
                    start=True, stop=True,
                )
                nc.scalar.activation(
                    zT[:, jh * 512 : (jh + 1) * 512], ps[:], AF.Identity, bias=benc[:]
                )
            zbT = const.tile([128, 128], F32, tag="zbT")
            ps = pp.tile([128, 128], F32, tag="tps")
            nc.tensor.matmul(ps[:], wenc[:], xbT[:], start=True, stop=True)
            nc.scalar.activation(zbT[:], ps[:], AF.Identity, bias=benc[:])

            # A^T chunks (fp16) and P = exp(A^T + ln(alpha)) (fp16)
            lnalpha = const.tile([128, 1], F32, tag="lnalpha")
            nc.vector.memset(lnalpha[:], float(np.log(ALPHA)))
            AT, Pt = [], []
            for c in range(2):
                at = const.tile([128, 1024], F16, tag=f"AT{c}")
                p = const.tile([128, 1024], F16, tag=f"P{c}")
                for jh in range(2):
                    ps = pp.tile([128, 512], F32, tag="zps")
                    nc.tensor.matmul(
                        ps[:], w1a[:, c * 128 : (c + 1) * 128],
                        zT[:, jh * 512 : (jh + 1) * 512],
                        start=True, stop=True,
                    )
                    sl = slice(jh * 512, (jh + 1) * 512)
                    nc.scalar.activation(at[:, sl], ps[:], AF.Copy)
                    nc.scalar.activation(
                        p[:, sl], ps[:], AF.Exp, bias=lnalpha[:]
                    )
                AT.append(at)
                Pt.append(p)

            # B'^T = W1b^T zb^T + b1 (fp32) and Q = exp(B'^T) (fp32), [128, IB]
            Bp, Qt = [], []
            for c in range(2):
                bp = const.tile([128, IB], F32, tag=f"Bp{c}")
                q = const.tile([128, IB], F32, tag=f"Q{c}")
                ps = pp.tile([128, IB], F32, tag="tps")
                nc.tensor.matmul(
                    ps[:], w1b[:, c * 128 : (c + 1) * 128], zbT[:],
                    start=True, stop=True,
                )
                nc.scalar.activation(bp[:], ps[:], AF.Identity, bias=b1t[c][:])
                nc.scalar.activation(q[:], ps[:], AF.Exp, bias=b1t[c][:])
                Bp.append(bp)
                Qt.append(q)

            # weight windows: zero [128,256] fp16 with col 128 = lam/6 * w2_c
            wwin = []
            for c in range(2):
                t = const.tile([128, 256], F16, tag=f"win{c}")
                nc.vector.memset(t[:], 0.0)
                nc.vector.tensor_scalar(
                    t[:, 128:129], w2t[:, c : c + 1], LAM / 6.0, None, OP.mult
                )
                wwin.append(t)

            # C vector: C = 0.5 + (b2 - lam*alpha*sum(W2))/6, broadcast [128,1]
            ones_col = const.tile([128, 1], F32, tag="ones_col")
            nc.vector.memset(ones_col[:], 1.0)
            ones_row = const.tile([1, 128], F32, tag="ones_row")
            nc.vector.memset(ones_row[:], 1.0)
            sps = pp1.tile([1, 1], F32, tag="sps")
            nc.tensor.matmul(sps[:], w2t[:, 0:1], ones_col[:], start=True, stop=False)
            nc.tensor.matmul(sps[:], w2t[:, 1:2], ones_col[:], start=False, stop=True)
            ssb = const.tile([1, 1], F32, tag="ssb")
            nc.vector.tensor_scalar(
                ssb[:], sps[:], -LAM * ALPHA / 6.0, None, OP.mult
            )
            s2 = const.tile([1, 1], F32, tag="s2")
            nc.vector.tensor_scalar(s2[:], b2t[:], 1.0 / 6.0, 0.5, OP.mult, OP.add)
            s3 = const.tile([1, 1], F32, tag="s3")
            nc.vector.tensor_add(s3[:], ssb[:], s2[:])
            cps = pp1.tile([128, 1], F32, tag="cps")
            nc.tensor.matmul(cps[:], ones_row[:], s3[:], start=True, stop=True)
            cvec = const.tile([128, 1], F32, tag="cvec")
            nc.vector.tensor_copy(cvec[:], cps[:])

        # ---------------- main loop --------------------------------------
        accA = accp.tile([128, 512], F32, tag="accA")
        accB = accp.tile([128, 512], F32, tag="accB")
        n_mm = {0: 0, 1: 0}  # per-bank matmul counter
        total_mm = n_i * 4  # per bank

        for i in range(n_i):
            pts = []
            for c in range(2):
                p1 = planes.tile([128, 1024], F16, tag=f"p1c{c}")
                if c == 0:
                    nc.scalar.activation(
                        p1[:], AT[c][:], AF.Relu, bias=Bp[c][:, i : i + 1]
                    )
                else:
                    nc.vector.tensor_scalar(
                        p1[:], AT[c][:], Bp[c][:, i : i + 1], 0.0, OP.add, OP.max
                    )
                p2 = planes.tile([128, 1024], F16, tag=f"p2c{c}")
                nc.vector.tensor_scalar(
                    p2[:], Pt[c][:], Qt[c][:, i : i + 1], float(ALPHA), OP.mult, OP.min
                )
                pts.append((p1, p2))
            for c in range(2):
                win = wwin[c][:, 128 - i : 256 - i]
                for p_ in pts[c]:
                    for bank, acc, sl in (
                        (0, accA, slice(0, 512)),
                        (1, accB, slice(512, 1024)),
                    ):
                        nc.tensor.matmul(
                            acc[:], win, p_[:, sl],
                            start=(n_mm[bank] == 0),
                            stop=(n_mm[bank] == total_mm - 1),
                            skip_group_check=True,
                        )
                        n_mm[bank] += 1

        # ---------------- epilogue ---------------------------------------
        outsb = const.tile([128, 1024], F32, tag="outsb")
        nc.scalar.activation(outsb[:, 0:512], accA[:], AF.Relu, bias=cvec[:])
        nc.scalar.activation(outsb[:, 512:1024], accB[:], AF.Relu, bias=cvec[:])
        outf = const.tile([128, 1024], F32, tag="outf")
        nc.vector.tensor_scalar(outf[:], outsb[:], 1.0, None, OP.min)
        nc.sync.dma_start(y_d[:, :], outf[:])

    nc.compile()
    return nc


def get_nc(n_i=IB):
    if n_i not in _CACHE:
        _CACHE[n_i] = build_kernel(n_i)
    return _CACHE[n_i]


def make_in_maps(inputs):
    x = np.ascontiguousarray(np.asarray(inputs["x"], dtype=np.float32))
    base = {
        "x": x,
        "w_enc": np.ascontiguousarray(np.asarray(inputs["W_enc"], np.float32)),
        "b_enc": np.asarray(inputs["b_enc"], np.float32).reshape(D, 1).copy(),
        "w1": np.ascontiguousarray(np.asarray(inputs["W1"], np.float32)),
        "b1": np.asarray(inputs["b1"], np.float32).reshape(H, 1).copy(),
        "w2": np.ascontiguousarray(np.asarray(inputs["W2"], np.float32)),
        "b2": np.asarray(inputs["b2"], np.float32).reshape(1, 1).copy(),
        "ident": np.eye(128, dtype=np.float32),
    }
    in_maps = []
    for g in range(N_CORES):
        m = dict(base)
        m["xb"] = np.ascontiguousarray(x[g * IB : (g + 1) * IB])
        in_maps.append(m)
    return in_maps


def run_on_cores(inputs, trace=False, **kwargs):
    nc = get_nc()
    in_maps = make_in_maps(inputs)
    res = bass_utils.run_bass_kernel_spmd(
        nc, in_maps, core_ids=list(range(N_CORES)), trace=trace, **kwargs
    )
    return res


def kernel(**inputs) -> np.ndarray:
    res = run_on_cores(inputs, trace=False)
    out = np.concatenate([res.results[g]["y"] for g in range(N_CORES)], axis=0)
    return out.astype(np.float32)
